# revision 4
# baseline (speedup 1.0000x reference)
"""Distributed Trainium2 kernel for attention-pooling.

Reference computation (B=4, S=4096, D=256, L=8921):
    scores = einsum('ld,bsd->bls', U, x)
    alpha  = softmax(scores, axis=2)            # over seq dim
    out    = einsum('bls,bsd->bld', alpha, x)
    return (out, alpha)

Sharding over 8 NeuronCores: grid = batch(4) x label-half(2).
Core c = b*2 + h computes batch b and labels [h*4608, (h+1)*4608)
(L padded 8921 -> 9216 = 2*4608 = 72 tiles of 128).

Per-core pipeline, per 128-label tile:
  - scores tile [128, 4096] via fp16 matmuls (PSUM f32 accumulate, K=256)
  - e = exp(scores - 40) in bf16 (constant shift instead of a row-max pass:
    scores ~ N(0, 256) so s-40 <= ~88 never overflows f32, and bf16's
    exponent range absorbs the cross-row spread of softmax numerators)
  - eT via one DMA xbar block-transpose  [128, 4096] -> [128, 32, 128]
  - pooled = eT.T @ [x | 1] accumulated over the 32 s-chunks; the appended
    ones column yields z = sum(e) for free
  - alpha = e * (1/z)  (f32, DMA'd out);  out = pooled * (1/z)
"""

import numpy as np

from concourse import bacc, tile
from concourse import mybir
from concourse.bass_utils import run_bass_kernel_spmd

B, S, D, L = 4, 4096, 256, 8921
P = 128
LSH = 4608          # labels per core (L padded to 9216 = 2*4608)
LT = LSH // P       # 36 label tiles per core
SJ = S // P         # 32 seq chunks of 128
SHIFT = 40.0

_NC_CACHE = {}


def build_kernel():
    f32 = mybir.dt.float32
    f16 = mybir.dt.float16
    bf16 = mybir.dt.bfloat16
    Exp = mybir.ActivationFunctionType.Exp

    nc = bacc.Bacc(None, target_bir_lowering=False)
    x_in = nc.declare_dram_parameter("x", [S, D], f32, isOutput=False)
    u_in = nc.declare_dram_parameter("u", [LSH, D], f32, isOutput=False)
    alpha_out = nc.declare_dram_parameter("alpha", [LSH, S], f32, isOutput=True)
    out_out = nc.declare_dram_parameter("out", [LSH, D], f32, isOutput=True)

    with tile.TileContext(nc) as tc:
        with tc.tile_pool(name="persist", bufs=1) as persist:
            # fp16 x^T, one tile per 128-wide k-chunk of D: xT[h][d', s]
            xT = [persist.tile([P, S], f16, name=f"xT{h}") for h in range(2)]
            # bf16 [x | 1] chunks for the pooled matmul: [s', j, d(+1)]
            x_aug = persist.tile([P, SJ, D + 1], bf16)
            # fp16 U^T stationary chunks: (lt, h) -> [:, (lt*2+h)*128 : +128]
            ut = persist.tile([P, LT * 2 * P], f16)
            # per-partition exp bias (constant shift)
            nbias = persist.tile([P, 1], f32)
            nc.vector.memset(nbias[:], -SHIFT)

            with tc.tile_pool(name="prep", bufs=1) as prep:
                xf = prep.tile([P, SJ, D], f32)
                for j in range(SJ):
                    nc.sync.dma_start(xf[:, j, :], x_in[j * P : (j + 1) * P, :])
                x16 = prep.tile([P, SJ, D], f16)
                nc.vector.tensor_copy(x16[:], xf[:])
                nc.vector.tensor_copy(x_aug[:, :, 0:D], xf[:])
                nc.vector.memset(x_aug[:, :, D : D + 1], 1.0)
                for j in range(SJ):
                    for h in range(2):
                        nc.sync.dma_start(
                            xT[h][:, j * P : (j + 1) * P],
                            x16[:, j, h * P : (h + 1) * P],
                            transpose=True,
                        )
                uf = prep.tile([P, LT, D], f32)
                for t in range(LT):
                    nc.sync.dma_start(uf[:, t, :], u_in[t * P : (t + 1) * P, :])
                u16 = prep.tile([P, LT, D], f16)
                nc.vector.tensor_copy(u16[:], uf[:])
                for t in range(LT):
                    for h in range(2):
                        c = (t * 2 + h) * P
                        nc.sync.dma_start(
                            ut[:, c : c + P],
                            u16[:, t, h * P : (h + 1) * P],
                            transpose=True,
                        )

            with (
                tc.tile_pool(name="ps", bufs=3, space="PSUM") as ps_pool,
                tc.tile_pool(name="po", bufs=2, space="PSUM") as po_pool,
                tc.tile_pool(name="e", bufs=4) as e_pool,
                tc.tile_pool(name="eT", bufs=2) as eT_pool,
                tc.tile_pool(name="al", bufs=3) as al_pool,
                tc.tile_pool(name="o", bufs=2) as o_pool,
                tc.tile_pool(name="st", bufs=4) as st_pool,
            ):

                def emit_scores(lt):
                    e_t = e_pool.tile([P, S], bf16)
                    l0 = lt * 2 * P
                    for q in range(4):  # 1024-wide psum chunks (2 banks)
                        ps = ps_pool.tile([P, 1024], f32)
                        for n in range(2):  # 512-wide matmuls (bank-aligned)
                            s0 = q * 1024 + n * 512
                            sl = ps[:, n * 512 : (n + 1) * 512]
                            nc.tensor.matmul(
                                sl, ut[:, l0 : l0 + P],
                                xT[0][:, s0 : s0 + 512],
                                start=True, stop=False,
                            )
                            nc.tensor.matmul(
                                sl, ut[:, l0 + P : l0 + 2 * P],
                                xT[1][:, s0 : s0 + 512],
                                start=False, stop=True,
                            )
                        nc.scalar.activation(
                            e_t[:, q * 1024 : (q + 1) * 1024], ps[:], Exp,
                            bias=nbias[:],
                        )
                    eT = eT_pool.tile([P, SJ, P], bf16)
                    nc.sync.dma_start(eT[:], e_t[:], transpose=True)
                    return e_t, eT

                def emit_pooled(lt, e_t, eT):
                    po = po_pool.tile([P, D + 1], f32)
                    for j in range(SJ):
                        nc.tensor.matmul(
                            po[:], eT[:, j, :], x_aug[:, j, :],
                            start=(j == 0), stop=(j == SJ - 1),
                        )
                    rz = st_pool.tile([P, 1], f32)
                    nc.vector.reciprocal(rz[:], po[:, D : D + 1])
                    o_t = o_pool.tile([P, D], f32)
                    nc.vector.tensor_scalar_mul(o_t[:], po[:, 0:D], rz[:])
                    nc.scalar.dma_start(
                        out_out[lt * P : (lt + 1) * P, :], o_t[:]
                    )
                    al = al_pool.tile([P, S], f32)
                    nc.vector.tensor_scalar_mul(al[:], e_t[:], rz[:])
                    nc.scalar.dma_start(
                        alpha_out[lt * P : (lt + 1) * P, :], al[:]
                    )

                prev = None
                for lt in range(LT):
                    cur = emit_scores(lt)
                    if prev is not None:
                        emit_pooled(lt - 1, *prev)
                    prev = cur
                emit_pooled(LT - 1, *prev)

    nc.compile()
    return nc


def _get_nc():
    if "nc" not in _NC_CACHE:
        _NC_CACHE["nc"] = build_kernel()
    return _NC_CACHE["nc"]


def run_sharded(x, U, trace=False):
    """x [B,S,D] f32, U [L,D] f32 -> (out, alpha), plus the raw result obj."""
    x = np.ascontiguousarray(np.asarray(x, dtype=np.float32))
    U = np.asarray(U, dtype=np.float32)
    Upad = np.zeros((2 * LSH, D), dtype=np.float32)
    Upad[:L] = U

    nc = _get_nc()
    in_maps = []
    for b in range(B):
        for h in range(2):
            in_maps.append(
                {"x": x[b], "u": np.ascontiguousarray(Upad[h * LSH : (h + 1) * LSH])}
            )
    res = run_bass_kernel_spmd(nc, in_maps, list(range(8)), trace=trace)

    out = np.empty((B, L, D), dtype=np.float32)
    alpha = np.empty((B, L, S), dtype=np.float32)
    for b in range(B):
        r0 = res.results[b * 2]
        r1 = res.results[b * 2 + 1]
        out[b, :LSH] = r0["out"]
        out[b, LSH:] = r1["out"][: L - LSH]
        alpha[b, :LSH] = r0["alpha"]
        alpha[b, LSH:] = r1["alpha"][: L - LSH]
    return (out, alpha), res


def kernel(x, U):
    return run_sharded(x, U)[0]


# revision 7
# speedup vs baseline: 1.3753x; 1.3753x over previous
"""Distributed Trainium2 kernel for attention-pooling.

Reference computation (B=4, S=4096, D=256, L=8921):
    scores = einsum('ld,bsd->bls', U, x)
    alpha  = softmax(scores, axis=2)            # over seq dim
    out    = einsum('bls,bsd->bld', alpha, x)
    return (out, alpha)

Sharding over 8 NeuronCores: grid = batch(4) x label-half(2).
Core c = b*2 + h computes batch b and labels [h*4608, (h+1)*4608)
(L padded 8921 -> 9216 = 2*4608 = 72 tiles of 128).

Per-core pipeline, per 128-label tile:
  - scores tile [128, 4096] via fp16 matmuls (PSUM f32 accumulate, K=256)
  - e = exp(scores - 40) in bf16 (constant shift instead of a row-max pass:
    scores ~ N(0, 256) so s-40 <= ~88 never overflows f32, and bf16's
    exponent range absorbs the cross-row spread of softmax numerators)
  - eT via one DMA xbar block-transpose  [128, 4096] -> [128, 32, 128]
  - pooled = eT.T @ [x | 1] accumulated over the 32 s-chunks; the appended
    ones column yields z = sum(e) for free
  - alpha = e * (1/z)  (f32, DMA'd out);  out = pooled * (1/z)
"""

import numpy as np

from concourse import bacc, tile
from concourse import mybir
from concourse.bass_utils import run_bass_kernel_spmd

B, S, D, L = 4, 4096, 256, 8921
P = 128
LSH = 4608          # labels per core (L padded to 9216 = 2*4608)
LT = LSH // P       # 36 label tiles per core
SJ = S // P         # 32 seq chunks of 128
SHIFT = 40.0

_NC_CACHE = {}


def build_kernel():
    f32 = mybir.dt.float32
    f16 = mybir.dt.float16
    bf16 = mybir.dt.bfloat16
    Exp = mybir.ActivationFunctionType.Exp

    nc = bacc.Bacc(None, target_bir_lowering=False)
    x_in = nc.declare_dram_parameter("x", [S, D], f32, isOutput=False)
    u_in = nc.declare_dram_parameter("u", [LSH, D], f32, isOutput=False)
    alpha_out = nc.declare_dram_parameter("alpha", [LSH, S], f32, isOutput=True)
    out_out = nc.declare_dram_parameter("out", [LSH, D], f32, isOutput=True)

    with tile.TileContext(nc) as tc:
        with tc.tile_pool(name="persist", bufs=1) as persist:
            # fp16 x^T, one tile per 128-wide k-chunk of D: xT[h][d', s]
            xT = [persist.tile([P, S], f16, name=f"xT{h}") for h in range(2)]
            # bf16 [x | 1] chunks for the pooled matmul: [s', j, d(+1)]
            x_aug = persist.tile([P, SJ, D + 1], bf16)
            # fp16 U^T stationary chunks: (lt, h) -> [:, (lt*2+h)*128 : +128]
            ut = persist.tile([P, LT * 2 * P], f16)
            # per-partition exp bias (constant shift)
            nbias = persist.tile([P, 1], f32)
            nc.vector.memset(nbias[:], -SHIFT)

            with tc.tile_pool(name="prep", bufs=1) as prep:
                xf = prep.tile([P, SJ, D], f32)
                nc.sync.dma_start(
                    xf[:], x_in[:].rearrange("(j p) d -> p j d", p=P)
                )
                x16 = prep.tile([P, SJ, D], f16)
                nc.vector.tensor_copy(x16[:], xf[:])
                nc.vector.tensor_copy(x_aug[:, :, 0:D], xf[:])
                nc.vector.memset(x_aug[:, :, D : D + 1], 1.0)
                # one-shot xbar block transpose of all of x16:
                # xTi[a, b, c] = x16_flat[c, b*128+a], so block b = (j, h)
                # interleaved; de-interleave into xT[h] with strided DVE copies
                xTi = prep.tile([P, 2 * SJ, P], f16)
                nc.sync.dma_start(xTi[:], x16[:], transpose=True)
                for h in range(2):
                    nc.vector.tensor_copy(
                        xT[h][:].rearrange("p (j c) -> p j c", c=P),
                        xTi[:, h::2, :],
                    )
                uf = prep.tile([P, LT, D], f32)
                nc.sync.dma_start(
                    uf[:], u_in[:].rearrange("(t p) d -> p t d", p=P)
                )
                u16 = prep.tile([P, LT, D], f16)
                nc.vector.tensor_copy(u16[:], uf[:])
                # ut blocks come out directly as (lt, h) pairs
                nc.sync.dma_start(
                    ut[:].rearrange("p (b c) -> p b c", c=P), u16[:], transpose=True
                )

            with (
                tc.tile_pool(name="ps", bufs=3, space="PSUM") as ps_pool,
                tc.tile_pool(name="po", bufs=2, space="PSUM") as po_pool,
                tc.tile_pool(name="e", bufs=5) as e_pool,
                tc.tile_pool(name="eT", bufs=3) as eT_pool,
                tc.tile_pool(name="al", bufs=4) as al_pool,
                tc.tile_pool(name="o", bufs=3) as o_pool,
                tc.tile_pool(name="st", bufs=6) as st_pool,
            ):

                def emit_scores(lt):
                    e_t = e_pool.tile([P, S], bf16)
                    l0 = lt * 2 * P
                    for q in range(4):  # 1024-wide psum chunks (2 banks)
                        ps = ps_pool.tile([P, 1024], f32)
                        for n in range(2):  # 512-wide matmuls (bank-aligned)
                            s0 = q * 1024 + n * 512
                            sl = ps[:, n * 512 : (n + 1) * 512]
                            nc.tensor.matmul(
                                sl, ut[:, l0 : l0 + P],
                                xT[0][:, s0 : s0 + 512],
                                start=True, stop=False,
                            )
                            nc.tensor.matmul(
                                sl, ut[:, l0 + P : l0 + 2 * P],
                                xT[1][:, s0 : s0 + 512],
                                start=False, stop=True,
                            )
                        nc.scalar.activation(
                            e_t[:, q * 1024 : (q + 1) * 1024], ps[:], Exp,
                            bias=nbias[:],
                        )
                    eT = eT_pool.tile([P, SJ, P], bf16)
                    nc.sync.dma_start(eT[:], e_t[:], transpose=True)
                    return e_t, eT

                def emit_pooled(lt, e_t, eT):
                    po = po_pool.tile([P, D + 1], f32)
                    for j in range(SJ):
                        nc.tensor.matmul(
                            po[:], eT[:, j, :], x_aug[:, j, :],
                            start=(j == 0), stop=(j == SJ - 1),
                        )
                    rz = st_pool.tile([P, 1], f32)
                    nc.vector.reciprocal(rz[:], po[:, D : D + 1])
                    o_t = o_pool.tile([P, D], f32)
                    nc.vector.tensor_scalar_mul(o_t[:], po[:, 0:D], rz[:])
                    nc.gpsimd.dma_start(
                        out_out[lt * P : (lt + 1) * P, :], o_t[:]
                    )
                    al = al_pool.tile([P, S], f32)
                    nc.vector.tensor_scalar_mul(al[:], e_t[:], rz[:])
                    nc.gpsimd.dma_start(
                        alpha_out[lt * P : (lt + 1) * P, :], al[:]
                    )

                LAG = 2  # scores run LAG tiles ahead of pooled
                pending = []
                for lt in range(LT):
                    pending.append((lt, emit_scores(lt)))
                    if len(pending) > LAG:
                        plt, args = pending.pop(0)
                        emit_pooled(plt, *args)
                for plt, args in pending:
                    emit_pooled(plt, *args)

    nc.compile()
    return nc


def _get_nc():
    if "nc" not in _NC_CACHE:
        _NC_CACHE["nc"] = build_kernel()
    return _NC_CACHE["nc"]


def run_sharded(x, U, trace=False):
    """x [B,S,D] f32, U [L,D] f32 -> (out, alpha), plus the raw result obj."""
    x = np.ascontiguousarray(np.asarray(x, dtype=np.float32))
    U = np.asarray(U, dtype=np.float32)
    Upad = np.zeros((2 * LSH, D), dtype=np.float32)
    Upad[:L] = U

    nc = _get_nc()
    in_maps = []
    for b in range(B):
        for h in range(2):
            in_maps.append(
                {"x": x[b], "u": np.ascontiguousarray(Upad[h * LSH : (h + 1) * LSH])}
            )
    res = run_bass_kernel_spmd(nc, in_maps, list(range(8)), trace=trace)

    out = np.empty((B, L, D), dtype=np.float32)
    alpha = np.empty((B, L, S), dtype=np.float32)
    for b in range(B):
        r0 = res.results[b * 2]
        r1 = res.results[b * 2 + 1]
        out[b, :LSH] = r0["out"]
        out[b, LSH:] = r1["out"][: L - LSH]
        alpha[b, :LSH] = r0["alpha"]
        alpha[b, LSH:] = r1["alpha"][: L - LSH]
    return (out, alpha), res


def kernel(x, U):
    return run_sharded(x, U)[0]


# revision 8
# speedup vs baseline: 1.6435x; 1.1951x over previous
"""Distributed Trainium2 kernel for attention-pooling.

Reference computation (B=4, S=4096, D=256, L=8921):
    scores = einsum('ld,bsd->bls', U, x)
    alpha  = softmax(scores, axis=2)            # over seq dim
    out    = einsum('bls,bsd->bld', alpha, x)
    return (out, alpha)

Sharding over 8 NeuronCores: grid = batch(4) x label-half(2).
Core c = b*2 + h computes batch b and labels [h*4608, (h+1)*4608)
(L padded 8921 -> 9216 = 2*4608 = 72 tiles of 128).

Per-core structure, per 128-label tile:
  - scores tile [128l, 4096s] via fp16 matmuls (PSUM f32, K=256)
  - e = exp(scores - 40) in bf16. The constant shift replaces a row-max
    pass: scores ~ N(0, 256) so exp(s-40) never overflows f32, and bf16's
    exponent range absorbs the cross-row spread of softmax numerators.
  - pooled = eT.T @ [x | 1] accumulated over 32 s-chunks; the ones column
    yields z = sum_s(e) for free -> alpha = e/z, out = pooled/z.
  - eT [s, l] comes from one of two paths (the work is split to balance
    the DMA engines against the TensorEngine):
      * DMA path: one xbar block-transpose of the e tile (fast to issue
        but pays a 256B-packet tax on the DMA engines)
      * sT path: recompute scores transposed on the PE (lhsT = xT chunk,
        rhs = UT 512-label slab) and exp straight into eT chunks; done in
        "super" blocks of 4 label tiles so the moving operand is 512 wide.
"""

import numpy as np

from concourse import bacc, tile
from concourse import mybir
from concourse.bass_utils import run_bass_kernel_spmd

B, S, D, L = 4, 4096, 256, 8921
P = 128
LSH = 4608          # labels per core (L padded to 9216 = 2*4608)
LT = LSH // P       # 36 label tiles per core
SJ = S // P         # 32 seq chunks of 128
SHIFT = 40.0
GROUP = 9           # tiles per schedule group
NSUP = 4            # leading tiles of each group use the sT path (one super)
NDMA = GROUP - NSUP

_NC_CACHE = {}


def build_kernel():
    f32 = mybir.dt.float32
    f16 = mybir.dt.float16
    bf16 = mybir.dt.bfloat16
    Exp = mybir.ActivationFunctionType.Exp

    nc = bacc.Bacc(None, target_bir_lowering=False)
    x_in = nc.declare_dram_parameter("x", [S, D], f32, isOutput=False)
    u_in = nc.declare_dram_parameter("u", [LSH, D], f32, isOutput=False)
    alpha_out = nc.declare_dram_parameter("alpha", [LSH, S], f32, isOutput=True)
    out_out = nc.declare_dram_parameter("out", [LSH, D], f32, isOutput=True)

    with tile.TileContext(nc) as tc:
        with tc.tile_pool(name="persist", bufs=1) as persist:
            # fp16 x^T, one tile per 128-wide k-chunk of D: xT[h][d', s]
            xT = [persist.tile([P, S], f16, name=f"xT{h}") for h in range(2)]
            # fp16 U^T, one tile per k-chunk: uth[h][d', l]
            uth = [persist.tile([P, LSH], f16, name=f"uth{h}") for h in range(2)]
            # bf16 [x | 1] chunks for the pooled matmul: [s', j, d(+1)]
            x_aug = persist.tile([P, SJ, D + 1], bf16)
            # per-partition exp bias (constant shift)
            nbias = persist.tile([P, 1], f32)
            nc.vector.memset(nbias[:], -SHIFT)

            with tc.tile_pool(name="prep", bufs=1) as prep:
                xf = prep.tile([P, SJ, D], f32)
                nc.sync.dma_start(
                    xf[:], x_in[:].rearrange("(j p) d -> p j d", p=P)
                )
                x16 = prep.tile([P, SJ, D], f16)
                nc.vector.tensor_copy(x16[:], xf[:])
                nc.vector.tensor_copy(x_aug[:, :, 0:D], xf[:])
                nc.vector.memset(x_aug[:, :, D : D + 1], 1.0)
                # one xbar block transpose of all of x16:
                # xTi[a, b, c] = x16_flat[c, b*128+a]  (block b = (j, h))
                xTi = prep.tile([P, 2 * SJ, P], f16)
                nc.sync.dma_start(xTi[:], x16[:], transpose=True)
                for h in range(2):
                    nc.vector.tensor_copy(
                        xT[h][:].rearrange("p (j c) -> p j c", c=P),
                        xTi[:, h::2, :],
                    )
                uf = prep.tile([P, LT, D], f32)
                nc.sync.dma_start(
                    uf[:], u_in[:].rearrange("(t p) d -> p t d", p=P)
                )
                u16 = prep.tile([P, LT, D], f16)
                nc.vector.tensor_copy(u16[:], uf[:])
                uti = prep.tile([P, 2 * LT, P], f16)
                nc.sync.dma_start(uti[:], u16[:], transpose=True)
                for h in range(2):
                    nc.vector.tensor_copy(
                        uth[h][:].rearrange("p (t c) -> p t c", c=P),
                        uti[:, h::2, :],
                    )

            with (
                tc.tile_pool(name="psum", bufs=1, space="PSUM") as ps_pool,
                tc.tile_pool(name="e", bufs=5) as e_pool,
                tc.tile_pool(name="eT", bufs=3) as eT_pool,
                tc.tile_pool(name="eTj", bufs=4) as eTj_pool,
                tc.tile_pool(name="al", bufs=4) as al_pool,
                tc.tile_pool(name="o", bufs=3) as o_pool,
                tc.tile_pool(name="st", bufs=6) as st_pool,
            ):

                def scores_ls(lt, e_t):
                    """[l, s] scores for one 128-label tile + exp into e_t."""
                    l0 = lt * P
                    for q in range(8):
                        sp = ps_pool.tile([P, 512], f32, name="sp", tag="sp", bufs=3)
                        s0 = q * 512
                        nc.tensor.matmul(
                            sp[:], uth[0][:, l0 : l0 + P],
                            xT[0][:, s0 : s0 + 512], start=True, stop=False,
                        )
                        nc.tensor.matmul(
                            sp[:], uth[1][:, l0 : l0 + P],
                            xT[1][:, s0 : s0 + 512], start=False, stop=True,
                        )
                        nc.scalar.activation(
                            e_t[:, s0 : s0 + 512], sp[:], Exp, bias=nbias[:]
                        )

                def epilogue(lt, po, e_t):
                    rz = st_pool.tile([P, 1], f32, name="rz")
                    nc.vector.reciprocal(rz[:], po[:, D : D + 1])
                    o_t = o_pool.tile([P, D], f32, name="o_t")
                    nc.vector.tensor_scalar_mul(o_t[:], po[:, 0:D], rz[:])
                    nc.gpsimd.dma_start(out_out[lt * P : (lt + 1) * P, :], o_t[:])
                    al = al_pool.tile([P, S], f32, name="al")
                    nc.vector.tensor_scalar_mul(al[:], e_t[:], rz[:])
                    nc.gpsimd.dma_start(alpha_out[lt * P : (lt + 1) * P, :], al[:])

                def super_block(g):
                    """Transposed scores + pooled for tiles g*9 .. g*9+3."""
                    l0 = g * GROUP * P
                    pos = [
                        ps_pool.tile([P, 512], f32, name=f"po_s{t}", tag="po", bufs=5)
                        for t in range(NSUP)
                    ]
                    prev = None
                    for j in range(SJ + 1):
                        if j < SJ:
                            sp = ps_pool.tile(
                                [P, 512], f32, name="spt", tag="sp", bufs=3
                            )
                            nc.tensor.matmul(
                                sp[:], xT[0][:, j * P : (j + 1) * P],
                                uth[0][:, l0 : l0 + NSUP * P],
                                start=True, stop=False,
                            )
                            nc.tensor.matmul(
                                sp[:], xT[1][:, j * P : (j + 1) * P],
                                uth[1][:, l0 : l0 + NSUP * P],
                                start=False, stop=True,
                            )
                            ej = eTj_pool.tile([P, NSUP * P], bf16, name="ej")
                            nc.scalar.activation(ej[:], sp[:], Exp, bias=nbias[:])
                            cur = (j, ej)
                        else:
                            cur = None
                        if prev is not None:
                            pj, pej = prev
                            for t in range(NSUP):
                                nc.tensor.matmul(
                                    pos[t][:, 0 : D + 1],
                                    pej[:, t * P : (t + 1) * P],
                                    x_aug[:, pj, :],
                                    start=(pj == 0), stop=(pj == SJ - 1),
                                )
                        prev = cur
                    return pos

                def sT_alpha(lt, po):
                    e_t = e_pool.tile([P, S], bf16, name="e_t")
                    scores_ls(lt, e_t)
                    epilogue(lt, po, e_t)

                def dma_scores(lt):
                    e_t = e_pool.tile([P, S], bf16, name="e_t")
                    scores_ls(lt, e_t)
                    eT = eT_pool.tile([P, SJ, P], bf16, name="eT")
                    nc.sync.dma_start(eT[:], e_t[:], transpose=True)
                    return e_t, eT

                def dma_pooled(lt, e_t, eT):
                    po = ps_pool.tile([P, 512], f32, name="po_d", tag="po", bufs=5)
                    for j in range(SJ):
                        nc.tensor.matmul(
                            po[:, 0 : D + 1], eT[:, j, :], x_aug[:, j, :],
                            start=(j == 0), stop=(j == SJ - 1),
                        )
                    epilogue(lt, po, e_t)

                deferred = None
                for g in range(LT // GROUP):
                    t0 = g * GROUP
                    pos = super_block(g)
                    if deferred is not None:
                        dma_pooled(*deferred)
                        deferred = None
                    sts = [t0 + i for i in range(NSUP)]
                    dts = [t0 + NSUP + i for i in range(NDMA)]
                    store = {}
                    seq = [
                        ("s", dts[0]), ("a", 0), ("s", dts[1]), ("a", 1),
                        ("p", dts[0]), ("s", dts[2]), ("a", 2), ("p", dts[1]),
                        ("s", dts[3]), ("a", 3), ("p", dts[2]), ("s", dts[4]),
                        ("p", dts[3]),
                    ]
                    for kind, idx in seq:
                        if kind == "s":
                            store[idx] = dma_scores(idx)
                        elif kind == "a":
                            sT_alpha(sts[idx], pos[idx])
                        else:
                            dma_pooled(idx, *store.pop(idx))
                    deferred = (dts[4], *store.pop(dts[4]))
                if deferred is not None:
                    dma_pooled(*deferred)

    nc.compile()
    return nc


def _get_nc():
    if "nc" not in _NC_CACHE:
        _NC_CACHE["nc"] = build_kernel()
    return _NC_CACHE["nc"]


def run_sharded(x, U, trace=False):
    """x [B,S,D] f32, U [L,D] f32 -> (out, alpha), plus the raw result obj."""
    x = np.ascontiguousarray(np.asarray(x, dtype=np.float32))
    U = np.asarray(U, dtype=np.float32)
    Upad = np.zeros((2 * LSH, D), dtype=np.float32)
    Upad[:L] = U

    nc = _get_nc()
    in_maps = []
    for b in range(B):
        for h in range(2):
            in_maps.append(
                {"x": x[b], "u": np.ascontiguousarray(Upad[h * LSH : (h + 1) * LSH])}
            )
    res = run_bass_kernel_spmd(nc, in_maps, list(range(8)), trace=trace)

    out = np.empty((B, L, D), dtype=np.float32)
    alpha = np.empty((B, L, S), dtype=np.float32)
    for b in range(B):
        r0 = res.results[b * 2]
        r1 = res.results[b * 2 + 1]
        out[b, :LSH] = r0["out"]
        out[b, LSH:] = r1["out"][: L - LSH]
        alpha[b, :LSH] = r0["alpha"]
        alpha[b, LSH:] = r1["alpha"][: L - LSH]
    return (out, alpha), res


def kernel(x, U):
    return run_sharded(x, U)[0]
